# revision 1
# baseline (speedup 1.0000x reference)
"""DeepseekV3 MLA attention on 8 trn2 NeuronCores.

Sharding: core c owns token shard (batch c//4, seq block c%4 of 512) for the
low-rank projections and the final o_proj, and heads {2c, 2c+1} for the
attention itself.  Communication: one AllGather of the compressed KV latent,
one AllToAll to redistribute q from token-shards to head-shards, and one
AllToAll to bring attention outputs back to token-shards before o_proj.
All matmuls run bf16 with fp32 PSUM accumulation.
"""

import contextlib
import math
import numpy as np
import ml_dtypes

import concourse.bacc as bacc
import concourse.bass as bass
import concourse.mybir as mybir
import concourse.tile as tile
from concourse import bass_utils

F32 = mybir.dt.float32
BF16 = mybir.dt.bfloat16
AF = mybir.ActivationFunctionType
ALU = mybir.AluOpType

# ---- model dims (hardcoded per problem spec) ----
B, S, HID = 2, 2048, 2048
H = 16
QLR, KVLR = 1536, 512
DN, DR, DV = 128, 64, 128
EPS = 1e-6
ROPE_BASE = 10000.0
SCALE = 1.0 / math.sqrt(DN + DR)

NCORES = 8
T = (B * S) // NCORES          # 512 tokens per core
NBLK = S // T                  # 4 seq blocks per batch
P = 128
QCOLS = H * (DN + DR)          # 3072
RB = QCOLS // NCORES           # 384 q rows per rank block (perm layout)
KVC = KVLR + DR                # 576
OB = 2 * DV                    # 256 rows per rank block of o_loc
AT = B * S                     # 4096 tokens total
NKH = HID // P                 # 16
NKQ = QLR // P                 # 12
NKV = KVLR // P                # 4

bf16 = ml_dtypes.bfloat16


# ----------------------------------------------------------------------------
# device program
# ----------------------------------------------------------------------------

def _rope(nc, pool, dst_bf, src, cos_sb, sin_sb, nheads):
    hw = DR // 2
    for h in range(nheads):
        x1 = src[h * DR: h * DR + hw, :]
        x2 = src[h * DR + hw: (h + 1) * DR, :]
        ta = pool.tile([hw, T], F32, name="rope_ta", tag="sq", bufs=2)
        tb = pool.tile([hw, T], F32, name="rope_tb", tag="sq", bufs=2)
        nc.vector.tensor_mul(ta[:], x1, cos_sb[:])
        nc.vector.tensor_mul(tb[:], x2, sin_sb[:])
        nc.vector.tensor_sub(dst_bf[h * DR: h * DR + hw, :], ta[:], tb[:])
        nc.vector.tensor_mul(ta[:], x1, sin_sb[:])
        nc.vector.tensor_mul(tb[:], x2, cos_sb[:])
        nc.vector.tensor_add(dst_bf[h * DR + hw: (h + 1) * DR, :], ta[:], tb[:])


def _dma_ktiles(nc, eng, wt, wsrc, nk, nchunks=4):
    step = max(1, (nk + nchunks - 1) // nchunks)
    for k0 in range(0, nk, step):
        k1 = min(nk, k0 + step)
        eng.dma_start(wt[:, k0:k1, :], wsrc[:, k0:k1, :])


def _build_program():
    nc = bacc.Bacc("TRN2", target_bir_lowering=False, debug=False,
                   num_devices=NCORES)

    def din(name, shape, dt=BF16):
        return nc.dram_tensor(name, list(shape), dt, kind="ExternalInput").ap()

    hT = din("hT", [HID, T])
    wqa = din("wqa", [HID, QLR])
    wqb = din("wqb", [QLR, QCOLS])
    wkva = din("wkva", [HID, KVC])
    wkvb = din("wkvb", [KVLR, 512])
    wo = din("wo", [H * DV, HID])
    cst = din("cst", [DR, T])                # bf16 cos/sin rows (own pos)
    masks = din("masks", [NBLK, P, T])
    # out is feature-major [HID, T]; host transposes
    out = nc.dram_tensor("out", [HID, T], F32, kind="ExternalOutput").ap()
    rg = [list(range(NCORES))]

    ctx = contextlib.ExitStack()
    with tile.TileContext(nc) as tc, ctx:
        dram = ctx.enter_context(tc.tile_pool(name="dram", bufs=1,
                                              space="DRAM"))
        kv_loc = dram.tile([KVC, T], BF16)
        kv_all = dram.tile([NCORES * KVC, T], BF16, addr_space="Shared")
        q_loc0 = dram.tile([NCORES * 192, T], BF16)
        q_loc1 = dram.tile([NCORES * 192, T], BF16)
        q_mine0 = dram.tile([NCORES * 192, T], BF16)
        q_mine1 = dram.tile([NCORES * 192, T], BF16)
        o_loc0 = dram.tile([NCORES * DV, T], BF16)
        o_loc1 = dram.tile([NCORES * DV, T], BF16)
        o_mine0 = dram.tile([NCORES * DV, T], BF16)
        o_mine1 = dram.tile([NCORES * DV, T], BF16)

        # ---- LEFT stack: long-lived pools --------------------------------
        const = ctx.enter_context(tc.tile_pool(name="const", bufs=1,
                                               side="left"))
        s4 = ctx.enter_context(tc.tile_pool(name="s4", bufs=1, side="left"))
        s4w = ctx.enter_context(tc.tile_pool(name="s4w", bufs=3,
                                             side="left"))
        s4t = ctx.enter_context(tc.tile_pool(name="s4t", bufs=1,
                                             side="left"))
        ps_att = ctx.enter_context(tc.tile_pool(name="ps_att", bufs=2,
                                                space="PSUM", side="left"))
        st_s2 = contextlib.ExitStack()
        s2 = st_s2.enter_context(tc.tile_pool(name="s2", bufs=1,
                                              side="left"))
        st_att = contextlib.ExitStack()
        att = st_att.enter_context(tc.tile_pool(name="att", bufs=1,
                                                side="left"))
        attq = st_att.enter_context(tc.tile_pool(name="attq", bufs=2,
                                                 side="left"))

        # ---- RIGHT stack: stage-1 pools (released mid-kernel) ------------
        st_s1w = contextlib.ExitStack()
        pool_qlr = st_s1w.enter_context(tc.tile_pool(name="pool_qlr",
                                                     bufs=1, side="right"))
        s1w = st_s1w.enter_context(tc.tile_pool(name="s1w", bufs=2,
                                                side="right"))
        s1wb = st_s1w.enter_context(tc.tile_pool(name="s1wb", bufs=3,
                                                 side="right"))
        s1t = st_s1w.enter_context(tc.tile_pool(name="s1t", bufs=1,
                                                side="right"))
        st_h = contextlib.ExitStack()
        ph = st_h.enter_context(tc.tile_pool(name="pool_h", bufs=1,
                                             side="right"))

        # ---- consts; hT first (PE needs it immediately) ------------------
        hT_sb = ph.tile([P, NKH, T], BF16)
        for k in range(NKH):
            nc.scalar.dma_start(hT_sb[:, k, :], hT[k * P:(k + 1) * P, :])
        ones_f = const.tile([P, 1], F32)
        nc.gpsimd.memset(ones_f[:], 1.0)
        ones_bf = const.tile([P, 1], BF16)
        nc.gpsimd.memset(ones_bf[:], 1.0)
        CC_sb = const.tile([P, T], BF16)   # [cos;cos;cos;cos] rows
        SS_sb = const.tile([P, T], BF16)
        hw = DR // 2
        for r in range(4):
            nc.scalar.dma_start(CC_sb[r * hw:(r + 1) * hw, :],
                                cst[0:hw, :])
            nc.scalar.dma_start(SS_sb[r * hw:(r + 1) * hw, :],
                                cst[hw:DR, :])
        cos_sb = CC_sb[0:hw, :]
        sin_sb = SS_sb[0:hw, :]
        mask_sb = const.tile([P, NBLK, T], BF16)
        for j in range(NBLK):
            nc.scalar.dma_start(mask_sb[:, j, :], masks[j])

        qlr_bf = pool_qlr.tile([P, NKQ, T], BF16)
        kv_bf = pool_qlr.tile([P, NKV, T], BF16)

        # ---------------- stage 1a: kv_a + rmsnorm + rope ------------------
        ss_kv = ps_att.tile([1, T], F32, name="ss_kv", tag="psd", bufs=2)
        kv_m_sizes = [P] * NKV + [DR]
        for m in range(NKV + 1):
            mw = kv_m_sizes[m]
            wt = s1w.tile([P, NKH, P], BF16, name="w_t", tag="w_t")
            wsrc = wkva[:, m * P: m * P + mw].rearrange("(k p) m -> p k m",
                                                        p=P)
            _dma_ktiles(nc, nc.sync, wt[:, :, :mw], wsrc, NKH)
            ps = ps_att.tile([P, T], F32, name="pss", tag="pss", bufs=4)
            for k in range(NKH):
                nc.tensor.matmul(ps[:mw, :], wt[:, k, :mw], hT_sb[:, k, :],
                                 start=(k == 0), stop=(k == NKH - 1))
            if m < NKV:
                nc.vector.tensor_copy(kv_bf[:, m, :], ps[:, :])
                sq = s1t.tile([P, T], F32, name="sq", tag="sq", bufs=2)
                nc.vector.tensor_mul(sq[:], kv_bf[:, m, :], kv_bf[:, m, :])
                nc.tensor.matmul(ss_kv[:], ones_f[:], sq[:],
                                 start=(m == 0), stop=(m == NKV - 1))
            else:
                kro_bf = s1t.tile([DR, T], BF16, name="kro_bf", tag="obf", bufs=2)
                _rope(nc, s1t, kro_bf[:], ps, cos_sb, sin_sb, 1)
                nc.scalar.dma_start(kv_loc[KVLR:KVC, :], kro_bf[:])

        tmp = s1t.tile([1, T], F32, name="tmp_kv", tag="tmp", bufs=1)
        nc.vector.tensor_scalar(tmp[:], ss_kv[:], 1.0 / KVLR, EPS,
                                op0=ALU.mult, op1=ALU.add)
        nc.scalar.activation(tmp[:], tmp[:], AF.Sqrt)
        nc.vector.reciprocal(tmp[:], tmp[:])
        rsb_kv = s1t.tile([P, T], F32, name="rsb_kv", tag="rsb", bufs=2)
        nc.gpsimd.partition_broadcast(rsb_kv[:], tmp[:])
        for m in range(NKV):
            ckv_bf = s1t.tile([P, T], BF16, name="ckv_bf", tag="obf", bufs=2)
            nc.vector.tensor_mul(ckv_bf[:], kv_bf[:, m, :], rsb_kv[:])
            nc.scalar.dma_start(kv_loc[m * P:(m + 1) * P, :], ckv_bf[:])

        nc.gpsimd.collective_compute(
            "AllGather", ALU.bypass, replica_groups=rg,
            ins=[kv_loc.opt()], outs=[kv_all.opt()])

        # ---------------- stage 1b: q_a + rmsnorm --------------------------
        ss_q = ps_att.tile([1, T], F32, name="ss_q", tag="psd", bufs=2)
        for m in range(NKQ):
            wt = s1w.tile([P, NKH, P], BF16, name="w_t", tag="w_t")
            wsrc = wqa[:, m * P:(m + 1) * P].rearrange("(k p) m -> p k m",
                                                       p=P)
            _dma_ktiles(nc, nc.sync, wt, wsrc, NKH)
            ps = ps_att.tile([P, T], F32, name="pss", tag="pss", bufs=4)
            for k in range(NKH):
                nc.tensor.matmul(ps[:], wt[:, k, :], hT_sb[:, k, :],
                                 start=(k == 0), stop=(k == NKH - 1))
            nc.vector.tensor_copy(qlr_bf[:, m, :], ps[:])
            sq = s1t.tile([P, T], F32, name="sq", tag="sq", bufs=2)
            nc.vector.tensor_mul(sq[:], qlr_bf[:, m, :], qlr_bf[:, m, :])
            nc.tensor.matmul(ss_q[:], ones_f[:], sq[:],
                             start=(m == 0), stop=(m == NKQ - 1))

        tmpq = s1t.tile([1, T], F32, name="tmp_q", tag="tmp", bufs=1)
        nc.vector.tensor_scalar(tmpq[:], ss_q[:], 1.0 / QLR, EPS,
                                op0=ALU.mult, op1=ALU.add)
        nc.scalar.activation(tmpq[:], tmpq[:], AF.Sqrt)
        nc.vector.reciprocal(tmpq[:], tmpq[:])
        rsb_q = s1t.tile([P, T], F32, name="rsb_q", tag="rsb", bufs=2)
        nc.gpsimd.partition_broadcast(rsb_q[:], tmpq[:])

        # ---- stage-2 SBUF + preloads (gpsimd SWDGE; waits on AG) --------
        wkvb_sb = s2.tile([P, NKV, 512], BF16)
        for k in range(NKV):
            nc.gpsimd.dma_start(wkvb_sb[:, k, :], wkvb[k * P:(k + 1) * P, :])


        # hT no longer needed
        st_h.close()

        ckv_sb = s2.tile([P, NKV, AT], BF16)
        for j in range(NCORES):
            for k in range(NKV):
                nc.gpsimd.dma_start(
                    ckv_sb[:, k, j * T:(j + 1) * T],
                    kv_all[j * KVC + k * P: j * KVC + (k + 1) * P, :])
        kro2 = s2.tile([P, AT], BF16)
        for j in range(NCORES):
            nc.gpsimd.dma_start(kro2[0:DR, j * T:(j + 1) * T],
                                kv_all[j * KVC + KVLR: j * KVC + KVC, :])
            nc.gpsimd.dma_start(kro2[DR:P, j * T:(j + 1) * T],
                                kv_all[j * KVC + KVLR: j * KVC + KVC, :])
        # ---------------- stage 1c: q_b (+rope), permuted ------------------
        # unnormalized q_lr feeds the matmul; the per-token norm scale is
        # multiplied into the outputs (commutes through matmul and rope).
        # m-tile order: even-head halves first so the first A2A fires early.
        def qb_tile(m):
            wt = s1wb.tile([P, NKQ, P], BF16, name="wqb_t", tag="wqb_t")
            wsrc = wqb[:, m * P:(m + 1) * P].rearrange(
                "(k p) m -> p k m", p=P)
            _dma_ktiles(nc, nc.sync, wt, wsrc, NKQ)
            ps = ps_att.tile([P, T], F32, name="pss", tag="pss", bufs=4)
            for k in range(NKQ):
                nc.tensor.matmul(ps[:], wt[:, k, :], qlr_bf[:, k, :],
                                 start=(k == 0), stop=(k == NKQ - 1))
            d = m // 3
            qb_bf = s1t.tile([P, T], BF16, name="qb_bf", tag="obf", bufs=2)
            if m % 3 == 2:
                hw = DR // 2
                xr = s1t.tile([P, T], F32, name="xr", tag="sq", bufs=2)
                for hh in range(2):
                    o = hh * DR
                    nc.vector.tensor_scalar_mul(
                        xr[o:o + hw, :], ps[o + hw:o + DR, :], -1.0)
                    nc.vector.tensor_copy(xr[o + hw:o + DR, :],
                                          ps[o:o + hw, :])
                t1 = s1t.tile([P, T], F32, name="t1", tag="rsb", bufs=2)
                nc.vector.tensor_mul(t1[:], ps[:], CC_sb[:])
                t2 = s1t.tile([P, T], F32, name="t2", tag="sq", bufs=2)
                nc.vector.tensor_mul(t2[:], xr[:], SS_sb[:])
                nc.vector.tensor_add(t1[:], t1[:], t2[:])
                nc.vector.tensor_mul(qb_bf[:], t1[:], rsb_q[:])
                nc.scalar.dma_start(
                    q_loc0[d * 192 + P: (d + 1) * 192, :], qb_bf[0:DR, :])
                nc.scalar.dma_start(
                    q_loc1[d * 192 + P: (d + 1) * 192, :], qb_bf[DR:P, :])
            else:
                nc.vector.tensor_mul(qb_bf[:], ps[:], rsb_q[:])
                qdst = q_loc0 if m % 3 == 0 else q_loc1
                nc.scalar.dma_start(qdst[d * 192: d * 192 + P, :], qb_bf[:])

        for d in range(NCORES):
            qb_tile(3 * d)
            qb_tile(3 * d + 2)
        nc.gpsimd.collective_compute(
            "AllToAll", ALU.bypass, replica_groups=rg,
            ins=[q_loc0.opt()], outs=[q_mine0.opt()])
        for d in range(NCORES):
            qb_tile(3 * d + 1)
        nc.gpsimd.collective_compute(
            "AllToAll", ALU.bypass, replica_groups=rg,
            ins=[q_loc1.opt()], outs=[q_mine1.opt()])

        # stage-1 pools done
        st_s1w.close()

        # ---------------- stage 2: expand kv_b -----------------------------
        kn_sb = s2.tile([P, 2, AT], BF16)
        for h in range(2):
            for jc in range(NCORES):
                ps = ps_att.tile([P, T], F32, name="pss", tag="pss", bufs=4)
                for k in range(NKV):
                    nc.tensor.matmul(
                        ps[:], wkvb_sb[:, k, h * P:(h + 1) * P],
                        ckv_sb[:, k, jc * T:(jc + 1) * T],
                        start=(k == 0), stop=(k == NKV - 1))
                nc.vector.tensor_copy(kn_sb[:, h, jc * T:(jc + 1) * T],
                                      ps[:])

        NVT = AT // P
        v_sb = s2.tile([P, NVT, OB], BF16)
        for t in range(NVT):
            ps = ps_att.tile([P, T], F32, name="pss", tag="pss", bufs=4)[:, :OB]
            for k in range(NKV):
                nc.tensor.matmul(
                    ps[:], ckv_sb[:, k, t * P:(t + 1) * P],
                    wkvb_sb[:, k, OB:2 * OB],
                    start=(k == 0), stop=(k == NKV - 1))
            nc.vector.tensor_copy(v_sb[:, t, :], ps[:])

        # ---------------- stage 3: attention -------------------------------
        om_sb = s4.tile([P, H, T], BF16)

        pending = [None]     # deferred PE consumer emission (SW pipeline)
        prs_hold = [None]    # bf16 probs accumulator across pair groups

        def flush_pending():
            if pending[0] is not None:
                pending[0]()
                pending[0] = None

        for h in range(2):
            o_loc_h = o_loc0 if h == 0 else o_loc1
            q_mine_h = q_mine0 if h == 0 else q_mine1
            for b in range(B):
                for qc in range(NBLK):
                    j = NBLK * b + qc
                    qn_sb = attq.tile([P, T], BF16, name="qn_sb")
                    nc.sync.dma_start(
                        qn_sb[:], q_mine_h[j * 192: j * 192 + P, :])
                    # load raw rope rows, rope on DVE, duplicate into both
                    # partition halves (second rope matmul runs in rows 64+)
                    qr2 = attq.tile([P, T], BF16, name="qr2", bufs=2)
                    nc.sync.dma_start(
                        qr2[0:DR, :],
                        q_mine_h[j * 192 + P: (j + 1) * 192, :])
                    nc.sync.dma_start(
                        qr2[DR:P, :],
                        q_mine_h[j * 192 + P: (j + 1) * 192, :])

                    pso = ps_att.tile([P, T], F32, name="pso", tag="pso")
                    psd = ps_att.tile([1, T], F32, name="psd", tag="psd",
                                      bufs=2)
                    nkp = 2 * (qc + 1)           # kt pairs
                    for kp in range(nkp):
                        kt0, kt1 = 2 * kp, 2 * kp + 1
                        gk0 = b * S + kt0 * P
                        gk1 = b * S + kt1 * P
                        pss_a = ps_att.tile([P, T], F32, name="pss",
                                            tag="pss", bufs=4)
                        pss_b = ps_att.tile([P, T], F32, name="pss",
                                            tag="pss", bufs=4)
                        nc.tensor.matmul(pss_a[:], kn_sb[:, h, gk0:gk0 + P],
                                         qn_sb[:], start=True, stop=False)
                        nc.tensor.matmul(pss_b[:], kn_sb[:, h, gk1:gk1 + P],
                                         qn_sb[:], start=True, stop=False)
                        # K=64 rope matmuls in different array row groups
                        nc.tensor.matmul(pss_a[:], kro2[0:DR, gk0:gk0 + P],
                                         qr2[0:DR, :], start=False,
                                         stop=True)
                        nc.tensor.matmul(pss_b[:], kro2[DR:P, gk1:gk1 + P],
                                         qr2[DR:P, :], start=False,
                                         stop=True)
                        # emit previous pair's PE consumers now — gives the
                        # ACT/DVE chain a full pair of slack
                        flush_pending()
                        pr_a = att.tile([P, T], BF16, name="pr", tag="pr",
                                        bufs=3)
                        nc.scalar.activation(pr_a[:], pss_a[:], AF.Exp,
                                             scale=SCALE)
                        dj0 = kt0 - NBLK * qc
                        if dj0 >= 0:
                            nc.vector.tensor_mul(pr_a[:], pr_a[:],
                                                 mask_sb[:, dj0, :])
                        pr_b = att.tile([P, T], BF16, name="pr", tag="pr",
                                        bufs=3)
                        nc.scalar.activation(pr_b[:], pss_b[:], AF.Exp,
                                             scale=SCALE)
                        dj1 = kt1 - NBLK * qc
                        if dj1 >= 0:
                            nc.vector.tensor_mul(pr_b[:], pr_b[:],
                                                 mask_sb[:, dj1, :])
                        if kp % 2 == 0:
                            prs = att.tile([P, T], BF16, name="prs",
                                           tag="prs", bufs=2)
                            nc.vector.tensor_add(prs[:], pr_a[:], pr_b[:])
                            prs_hold[0] = prs
                        else:
                            prs = prs_hold[0]
                            nc.vector.tensor_add(prs[:], prs[:], pr_a[:])
                            nc.vector.tensor_add(prs[:], prs[:], pr_b[:])

                        def consume(pr_a=pr_a, pr_b=pr_b, prs=prs, kp=kp,
                                    nkp=nkp, b=b, kt0=kt0, h=h, pso=pso,
                                    psd=psd, j=j, o_loc_h=o_loc_h):
                            if kp % 2 == 1:
                                nc.tensor.matmul(psd[:], ones_bf[:],
                                                 prs[:],
                                                 start=(kp == 1),
                                                 stop=(kp == nkp - 1))
                            vt0 = (b * S) // P + kt0
                            nc.tensor.matmul(
                                pso[:], v_sb[:, vt0, h * DV:(h + 1) * DV],
                                pr_a[:], start=(kp == 0), stop=False)
                            nc.tensor.matmul(
                                pso[:],
                                v_sb[:, vt0 + 1, h * DV:(h + 1) * DV],
                                pr_b[:], start=False, stop=(kp == nkp - 1))
                            if kp == nkp - 1:
                                den_row = att.tile([1, T], F32,
                                                   name="den_row",
                                                   tag="rec", bufs=2)
                                nc.scalar.copy(den_row[:], psd[:])
                                den_sq = att.tile([P, T // P], F32,
                                                  name="den_sq",
                                                  tag="den_sq", bufs=2)
                                nc.sync.dma_start(den_sq[:], den_row[:])
                                nc.vector.reciprocal(den_sq[:], den_sq[:])
                                rec = att.tile([1, T], F32, name="rec",
                                               tag="rec", bufs=2)
                                nc.sync.dma_start(rec[:], den_sq[:])
                                recb = att.tile([P, T], F32, name="recb",
                                                tag="recb", bufs=1)
                                nc.gpsimd.partition_broadcast(recb[:],
                                                              rec[:])
                                o_bf = att.tile([P, T], BF16, name="o_bf",
                                                tag="o_bf", bufs=1)
                                nc.vector.tensor_mul(o_bf[:], pso[:],
                                                     recb[:])
                                nc.sync.dma_start(
                                    o_loc_h[j * DV:(j + 1) * DV, :],
                                    o_bf[:])

                        pending[0] = consume

            flush_pending()
            # per-head A2A; the h0 one overlaps h1 attention
            o_mine_h = o_mine0 if h == 0 else o_mine1
            nc.gpsimd.collective_compute(
                "AllToAll", ALU.bypass, replica_groups=rg,
                ins=[(o_loc0 if h == 0 else o_loc1).opt()],
                outs=[o_mine_h.opt()])
            for g in range(NCORES):      # global head 2g+h
                nc.gpsimd.dma_start(om_sb[:, 2 * g + h, :],
                                    o_mine_h[g * P:(g + 1) * P, :])

        st_att.close()
        st_s2.close()

        # ---------------- stage 4: o_proj (transposed output) --------------
        # group A (first 8 hid tiles): even-head contraction runs in the
        # A2A#2 shadow (only needs o_mine0); odd heads finish after.
        NA = 6
        psA = []
        for i in range(NA):
            if i < 4:
                ps = ps_att.tile([P, T], F32, name=f"opA{i}", tag="pss",
                                 bufs=4)
            elif i < 6:
                ps = ps_att.tile([P, T], F32, name=f"opA{i}", tag="pso")
            else:
                ps = ps_att.tile([P, T], F32, name=f"opA{i}", tag="psd",
                                 bufs=2)
            psA.append(ps)
        wA = []
        for mh in range(NA):
            wt = s4w.tile([P, NKH, P], BF16, name="wo_t", tag="wo_t",
                          bufs=7)
            wsrc = wo[:, mh * P:(mh + 1) * P].rearrange(
                "(k p) m -> p k m", p=P)
            _dma_ktiles(nc, nc.sync, wt, wsrc, NKH)
            wA.append(wt)
            for ki, k in enumerate(range(0, H, 2)):   # even heads
                nc.tensor.matmul(psA[mh][:], wA[mh][:, k, :],
                                 om_sb[:, k, :], start=(ki == 0),
                                 stop=False)

        # odd halves + eviction (runs once the odd om tiles land)
        for mh in range(NA):
            for ki, k in enumerate(range(1, H, 2)):   # odd heads
                nc.tensor.matmul(psA[mh][:], wA[mh][:, k, :],
                                 om_sb[:, k, :], start=False,
                                 stop=(ki == 7))
            ot = s4t.tile([P, T], F32, name="ot")
            nc.vector.tensor_copy(ot[:], psA[mh][:])
            nc.sync.dma_start(out[mh * P:(mh + 1) * P, :], ot[:])

        # group B: remaining hid tiles, full contraction
        for mh in range(NA, H):
            wt = s4w.tile([P, NKH, P], BF16, name="wo_t", tag="wo_t",
                          bufs=7)
            wsrc = wo[:, mh * P:(mh + 1) * P].rearrange(
                "(k p) m -> p k m", p=P)
            _dma_ktiles(nc, nc.sync, wt, wsrc, NKH)
            ps = ps_att.tile([P, T], F32, name="pss", tag="pss", bufs=4)
            for k in range(NKH):
                nc.tensor.matmul(ps[:], wt[:, k, :], om_sb[:, k, :],
                                 start=(k == 0), stop=(k == NKH - 1))
            ot = s4t.tile([P, T], F32, name="ot")
            nc.vector.tensor_copy(ot[:], ps[:])
            nc.sync.dma_start(out[mh * P:(mh + 1) * P, :], ot[:])

    nc.compile()
    return nc


# ----------------------------------------------------------------------------
# host side: shard prep, run, gather
# ----------------------------------------------------------------------------

def _prep_in_maps(hidden_states, wq_a, gq_a, wq_b, wkv_a, gkv_a, wkv_b, wo):
    hidden_states = np.asarray(hidden_states, dtype=np.float32)
    wq_a = np.asarray(wq_a, np.float32)
    wq_b = np.asarray(wq_b, np.float32) * np.asarray(gq_a, np.float32)[:, None]
    wkv_a = np.asarray(wkv_a, np.float32)
    wkv_b = (np.asarray(wkv_b, np.float32)
             * np.asarray(gkv_a, np.float32)[:, None])
    wo = np.asarray(wo, np.float32)

    # permute wq_b columns into per-rank blocks
    perm = []
    for j in range(NCORES):
        for h in (2 * j, 2 * j + 1):
            perm.extend(range(h * DN, (h + 1) * DN))
        for h in (2 * j, 2 * j + 1):
            perm.extend(range(H * DN + h * DR, H * DN + (h + 1) * DR))
    wqb_perm = np.ascontiguousarray(wq_b[:, perm]).astype(bf16)

    wqa_b = wq_a.astype(bf16)
    wkva_b = wkv_a.astype(bf16)
    wo_b = wo.astype(bf16)

    inv_freq = 1.0 / (ROPE_BASE ** (np.arange(0, DR, 2, dtype=np.float32)
                                    / DR))
    masks = np.zeros((NBLK, P, T), np.float32)
    kk = np.arange(P)[:, None]
    qq = np.arange(T)[None, :]
    for jp in range(NBLK):
        masks[jp] = (P * jp + kk <= qq).astype(np.float32)
    masks_b = masks.astype(bf16)
    in_maps = []
    for c in range(NCORES):
        b, blk = divmod(c, NBLK)
        h0 = 2 * c
        tok0 = blk * T
        hT = np.ascontiguousarray(
            hidden_states[b, tok0:tok0 + T, :].T).astype(bf16)
        pos = np.arange(tok0, tok0 + T, dtype=np.float32)
        ang = inv_freq[:, None] * pos[None, :]
        cosT = np.cos(ang).astype(np.float32)
        sinT = np.sin(ang).astype(np.float32)
        cols = []
        for h in (h0, h0 + 1):
            cols.append(wkv_b[:, h * (DN + DV): h * (DN + DV) + DN])
        for h in (h0, h0 + 1):
            cols.append(wkv_b[:, h * (DN + DV) + DN: (h + 1) * (DN + DV)])
        wkvb_c = np.ascontiguousarray(np.concatenate(cols, 1)).astype(bf16)
        in_maps.append({
            "hT": hT, "wqa": wqa_b, "wqb": wqb_perm, "wkva": wkva_b,
            "wkvb": wkvb_c, "wo": wo_b,
            "cst": np.concatenate([cosT, sinT], 0).astype(bf16),
            "masks": masks_b,
        })
    return in_maps


_NC_CACHE = {}


def _install_profile_hook():
    """The agent image's antenv lacks axon_hooks; recreate it so
    run_bass_kernel_spmd(trace=True) can capture NTFF profiles."""
    import sys
    import types
    if "antenv.axon_hooks" in sys.modules:
        return
    mod = types.ModuleType("antenv.axon_hooks")
    mod._hook = None

    def set_axon_ntff_profile_hook(h):
        mod._hook = h

    def get_axon_ntff_profile_hook():
        return mod._hook

    mod.set_axon_ntff_profile_hook = set_axon_ntff_profile_hook
    mod.get_axon_ntff_profile_hook = get_axon_ntff_profile_hook
    sys.modules["antenv.axon_hooks"] = mod
    try:
        import antenv
        antenv.axon_hooks = mod
        from trn_agent_boot.trn_boot import _ntff_profile_via_ctypes
        hook = _ntff_profile_via_ctypes("/opt/axon/libaxon_pjrt.so")
        if hook is not None:
            mod._hook = hook
    except Exception as e:  # degrade to no tracing
        print(f"profile hook install failed: {e}")


def _get_nc():
    if "nc" not in _NC_CACHE:
        _NC_CACHE["nc"] = _build_program()
    return _NC_CACHE["nc"]


def run(inputs, trace=False):
    if trace:
        _install_profile_hook()
    nc = _get_nc()
    in_maps = _prep_in_maps(**inputs)
    res = bass_utils.run_bass_kernel_spmd(
        nc, in_maps, core_ids=list(range(NCORES)), trace=trace)
    full = np.zeros((B, S, HID), np.float32)
    for c in range(NCORES):
        b, blk = divmod(c, NBLK)
        full[b, blk * T:(blk + 1) * T, :] = res.results[c]["out"].T
    return full, res


def kernel(**inputs) -> np.ndarray:
    full, _ = run(inputs, trace=False)
    return full



# revision 14
# speedup vs baseline: 1.0041x; 1.0041x over previous
"""DeepseekV3 MLA attention on 8 trn2 NeuronCores.

Sharding: core c owns token shard (batch c//4, seq block c%4 of 512) for the
low-rank projections and the final o_proj, and heads {2c, 2c+1} for the
attention itself.  Communication: one AllGather of the compressed KV latent,
one AllToAll to redistribute q from token-shards to head-shards, and one
AllToAll to bring attention outputs back to token-shards before o_proj.
All matmuls run bf16 with fp32 PSUM accumulation.
"""

import contextlib
import math
import numpy as np
import ml_dtypes

import concourse.bacc as bacc
import concourse.bass as bass
import concourse.mybir as mybir
import concourse.tile as tile
from concourse import bass_utils

F32 = mybir.dt.float32
BF16 = mybir.dt.bfloat16
AF = mybir.ActivationFunctionType
ALU = mybir.AluOpType

# ---- model dims (hardcoded per problem spec) ----
B, S, HID = 2, 2048, 2048
H = 16
QLR, KVLR = 1536, 512
DN, DR, DV = 128, 64, 128
EPS = 1e-6
ROPE_BASE = 10000.0
SCALE = 1.0 / math.sqrt(DN + DR)

NCORES = 8
T = (B * S) // NCORES          # 512 tokens per core
NBLK = S // T                  # 4 seq blocks per batch
P = 128
QCOLS = H * (DN + DR)          # 3072
RB = QCOLS // NCORES           # 384 q rows per rank block (perm layout)
KVC = KVLR + DR                # 576
OB = 2 * DV                    # 256 rows per rank block of o_loc
AT = B * S                     # 4096 tokens total
NKH = HID // P                 # 16
NKQ = QLR // P                 # 12
NKV = KVLR // P                # 4

bf16 = ml_dtypes.bfloat16


# ----------------------------------------------------------------------------
# device program
# ----------------------------------------------------------------------------

def _rope(nc, pool, dst_bf, src, cos_sb, sin_sb, nheads):
    hw = DR // 2
    for h in range(nheads):
        x1 = src[h * DR: h * DR + hw, :]
        x2 = src[h * DR + hw: (h + 1) * DR, :]
        ta = pool.tile([hw, T], F32, name="rope_ta", tag="sq", bufs=2)
        tb = pool.tile([hw, T], F32, name="rope_tb", tag="sq", bufs=2)
        nc.vector.tensor_mul(ta[:], x1, cos_sb[:])
        nc.vector.tensor_mul(tb[:], x2, sin_sb[:])
        nc.vector.tensor_sub(dst_bf[h * DR: h * DR + hw, :], ta[:], tb[:])
        nc.vector.tensor_mul(ta[:], x1, sin_sb[:])
        nc.vector.tensor_mul(tb[:], x2, cos_sb[:])
        nc.vector.tensor_add(dst_bf[h * DR + hw: (h + 1) * DR, :], ta[:], tb[:])


def _build_program():
    nc = bacc.Bacc("TRN2", target_bir_lowering=False, debug=False,
                   num_devices=NCORES)

    def din(name, shape, dt=BF16):
        return nc.dram_tensor(name, list(shape), dt, kind="ExternalInput").ap()

    hT = din("hT", [HID, T])
    # weights host-pre-tiled: [m_tile, p, k_tile, m_cols] contiguous so each
    # SBUF weight tile is a single contiguous DMA
    wqa = din("wqa", [NKQ, P, NKH, P])
    wqb = din("wqb", [QCOLS // P, P, NKQ, P])
    wkva = din("wkva", [5, P, NKH, P])
    wkvb = din("wkvb", [P, NKV, 512])
    wo = din("wo", [NKH, P, NKH, P])
    cst = din("cst", [DR, T])                # bf16 cos/sin rows (own pos)
    masks = din("masks", [NBLK, P, T])
    # out is feature-major [HID, T]; host transposes
    out = nc.dram_tensor("out", [HID, T], F32, kind="ExternalOutput").ap()
    rg = [list(range(NCORES))]

    ctx = contextlib.ExitStack()
    with tile.TileContext(nc) as tc, ctx:
        dram = ctx.enter_context(tc.tile_pool(name="dram", bufs=1,
                                              space="DRAM"))
        kv_loc = dram.tile([KVC, T], BF16)
        kv_all = dram.tile([NCORES * KVC, T], BF16, addr_space="Shared")
        q_loc0 = dram.tile([NCORES * 192, T], BF16)
        q_loc1 = dram.tile([NCORES * 192, T], BF16)
        q_mine0 = dram.tile([NCORES * 192, T], BF16)
        q_mine1 = dram.tile([NCORES * 192, T], BF16)
        o_loc0 = dram.tile([NCORES * DV, T], BF16)
        o_loc1 = dram.tile([NCORES * DV, T], BF16)
        o_mine0 = dram.tile([NCORES * DV, T], BF16)
        o_mine1 = dram.tile([NCORES * DV, T], BF16)

        # ---- LEFT stack: long-lived pools --------------------------------
        const = ctx.enter_context(tc.tile_pool(name="const", bufs=1,
                                               side="left"))
        s4 = ctx.enter_context(tc.tile_pool(name="s4", bufs=1, side="left"))
        s4w = ctx.enter_context(tc.tile_pool(name="s4w", bufs=3,
                                             side="left"))
        s4t = ctx.enter_context(tc.tile_pool(name="s4t", bufs=1,
                                             side="left"))
        ps_att = ctx.enter_context(tc.tile_pool(name="ps_att", bufs=2,
                                                space="PSUM", side="left"))
        st_s2 = contextlib.ExitStack()
        s2 = st_s2.enter_context(tc.tile_pool(name="s2", bufs=1,
                                              side="left"))
        st_att = contextlib.ExitStack()
        att = st_att.enter_context(tc.tile_pool(name="att", bufs=1,
                                                side="left"))
        attq = st_att.enter_context(tc.tile_pool(name="attq", bufs=2,
                                                 side="left"))

        # ---- RIGHT stack: stage-1 pools (released mid-kernel) ------------
        st_s1w = contextlib.ExitStack()
        pool_qlr = st_s1w.enter_context(tc.tile_pool(name="pool_qlr",
                                                     bufs=1, side="right"))
        s1w = st_s1w.enter_context(tc.tile_pool(name="s1w", bufs=2,
                                                side="right"))
        s1wb = st_s1w.enter_context(tc.tile_pool(name="s1wb", bufs=2,
                                                 side="right"))
        s1t = st_s1w.enter_context(tc.tile_pool(name="s1t", bufs=1,
                                                side="right"))
        st_h = contextlib.ExitStack()
        ph = st_h.enter_context(tc.tile_pool(name="pool_h", bufs=1,
                                             side="right"))

        # ---- consts; hT first (PE needs it immediately) ------------------
        hT_sb = ph.tile([P, NKH, T], BF16)
        for k in range(NKH):
            nc.scalar.dma_start(hT_sb[:, k, :], hT[k * P:(k + 1) * P, :])
        ones_bf = const.tile([P, 1], BF16)
        nc.gpsimd.memset(ones_bf[:], 1.0)
        CC_sb = const.tile([P, T], BF16)   # [cos;cos;cos;cos] rows
        SS_sb = const.tile([P, T], BF16)
        hw = DR // 2
        for r in range(4):
            nc.scalar.dma_start(CC_sb[r * hw:(r + 1) * hw, :],
                                cst[0:hw, :])
            nc.scalar.dma_start(SS_sb[r * hw:(r + 1) * hw, :],
                                cst[hw:DR, :])
        cos_sb = CC_sb[0:hw, :]
        sin_sb = SS_sb[0:hw, :]
        mask_sb = const.tile([P, NBLK, T], BF16)
        for j in range(NBLK):
            nc.scalar.dma_start(mask_sb[:, j, :], masks[j])

        qlr_bf = pool_qlr.tile([P, NKQ, T], BF16)
        kv_bf = pool_qlr.tile([P, NKV, T], BF16)

        # wkvb is tiny and dep-free: load early on the scalar queue
        wkvb_sb = s2.tile([P, NKV, 512], BF16)
        nc.scalar.dma_start(wkvb_sb[:], wkvb)

        # ---------------- stage 1a: kv_a + rmsnorm + rope ------------------
        ss_kv = ps_att.tile([1, T], F32, name="ss_kv", tag="psd", bufs=2)
        kv_m_sizes = [P] * NKV + [DR]
        for m in range(NKV + 1):
            mw = kv_m_sizes[m]
            wt = s1w.tile([P, NKH, P], BF16, name="w_t", tag="w_t")
            nc.sync.dma_start(wt[:], wkva[m])
            ps = ps_att.tile([P, T], F32, name="pss", tag="pss", bufs=4)
            for k in range(NKH):
                nc.tensor.matmul(ps[:mw, :], wt[:, k, :mw], hT_sb[:, k, :],
                                 start=(k == 0), stop=(k == NKH - 1))
            if m < NKV:
                nc.vector.tensor_copy(kv_bf[:, m, :], ps[:, :])
                sq = s1t.tile([P, T], BF16, name="sq", tag="sq", bufs=2)
                nc.vector.tensor_mul(sq[:], kv_bf[:, m, :], kv_bf[:, m, :])
                nc.tensor.matmul(ss_kv[:], ones_bf[:], sq[:],
                                 start=(m == 0), stop=(m == NKV - 1))
            else:
                kro_bf = s1t.tile([DR, T], BF16, name="kro_bf", tag="obf", bufs=2)
                _rope(nc, s1t, kro_bf[:], ps, cos_sb, sin_sb, 1)
                nc.scalar.dma_start(kv_loc[KVLR:KVC, :], kro_bf[:])

        tmp = s1t.tile([1, T], F32, name="tmp_kv", tag="tmp", bufs=1)
        nc.vector.tensor_scalar(tmp[:], ss_kv[:], 1.0 / KVLR, EPS,
                                op0=ALU.mult, op1=ALU.add)
        nc.scalar.activation(tmp[:], tmp[:], AF.Sqrt)
        nc.vector.reciprocal(tmp[:], tmp[:])
        rsb_kv = s1t.tile([P, T], F32, name="rsb_kv", tag="rsb", bufs=2)
        nc.gpsimd.partition_broadcast(rsb_kv[:], tmp[:])
        for m in range(NKV):
            ckv_bf = s1t.tile([P, T], BF16, name="ckv_bf", tag="obf", bufs=2)
            nc.vector.tensor_mul(ckv_bf[:], kv_bf[:, m, :], rsb_kv[:])
            nc.scalar.dma_start(kv_loc[m * P:(m + 1) * P, :], ckv_bf[:])

        nc.gpsimd.collective_compute(
            "AllGather", ALU.bypass, replica_groups=rg,
            ins=[kv_loc.opt()], outs=[kv_all.opt()])

        # ---------------- stage 1b: q_a + rmsnorm --------------------------
        ss_q = ps_att.tile([1, T], F32, name="ss_q", tag="psd", bufs=2)
        for m in range(NKQ):
            wt = s1w.tile([P, NKH, P], BF16, name="w_t", tag="w_t")
            nc.sync.dma_start(wt[:], wqa[m])
            ps = ps_att.tile([P, T], F32, name="pss", tag="pss", bufs=4)
            for k in range(NKH):
                nc.tensor.matmul(ps[:], wt[:, k, :], hT_sb[:, k, :],
                                 start=(k == 0), stop=(k == NKH - 1))
            nc.vector.tensor_copy(qlr_bf[:, m, :], ps[:])
            sq = s1t.tile([P, T], BF16, name="sq", tag="sq", bufs=2)
            nc.vector.tensor_mul(sq[:], qlr_bf[:, m, :], qlr_bf[:, m, :])
            nc.tensor.matmul(ss_q[:], ones_bf[:], sq[:],
                             start=(m == 0), stop=(m == NKQ - 1))

        tmpq = s1t.tile([1, T], F32, name="tmp_q", tag="tmp", bufs=1)
        nc.vector.tensor_scalar(tmpq[:], ss_q[:], 1.0 / QLR, EPS,
                                op0=ALU.mult, op1=ALU.add)
        nc.scalar.activation(tmpq[:], tmpq[:], AF.Sqrt)
        nc.vector.reciprocal(tmpq[:], tmpq[:])
        rsb_q = s1t.tile([P, T], F32, name="rsb_q", tag="rsb", bufs=2)
        nc.gpsimd.partition_broadcast(rsb_q[:], tmpq[:])

        # hT no longer needed
        st_h.close()

        # ---------------- stage 1c: q_b (+rope), permuted ------------------
        # unnormalized q_lr feeds the matmul; the per-token norm scale is
        # multiplied into the outputs (commutes through matmul and rope).
        # m-tile order: even-head halves first so the first A2A fires early.
        def qb_tile(m):
            wt = s1wb.tile([P, NKQ, P], BF16, name="wqb_t", tag="wqb_t")
            nc.sync.dma_start(wt[:], wqb[m])
            ps = ps_att.tile([P, T], F32, name="pss", tag="pss", bufs=4)
            for k in range(NKQ):
                nc.tensor.matmul(ps[:], wt[:, k, :], qlr_bf[:, k, :],
                                 start=(k == 0), stop=(k == NKQ - 1))
            d = m // 3
            qb_bf = s1t.tile([P, T], BF16, name="qb_bf", tag="obf", bufs=2)
            if m % 3 == 2:
                hw = DR // 2
                xr = s1t.tile([P, T], F32, name="xr", tag="sq", bufs=2)
                for hh in range(2):
                    o = hh * DR
                    nc.vector.tensor_scalar_mul(
                        xr[o:o + hw, :], ps[o + hw:o + DR, :], -1.0)
                    nc.vector.tensor_copy(xr[o + hw:o + DR, :],
                                          ps[o:o + hw, :])
                t1 = s1t.tile([P, T], F32, name="t1", tag="rsb", bufs=2)
                nc.vector.tensor_mul(t1[:], ps[:], CC_sb[:])
                t2 = s1t.tile([P, T], F32, name="t2", tag="sq", bufs=2)
                nc.vector.tensor_mul(t2[:], xr[:], SS_sb[:])
                nc.vector.tensor_add(t1[:], t1[:], t2[:])
                nc.vector.tensor_mul(qb_bf[:], t1[:], rsb_q[:])
                nc.scalar.dma_start(
                    q_loc0[d * 192 + P: (d + 1) * 192, :], qb_bf[0:DR, :])
                nc.scalar.dma_start(
                    q_loc1[d * 192 + P: (d + 1) * 192, :], qb_bf[DR:P, :])
            else:
                nc.vector.tensor_mul(qb_bf[:], ps[:], rsb_q[:])
                qdst = q_loc0 if m % 3 == 0 else q_loc1
                nc.scalar.dma_start(qdst[d * 192: d * 192 + P, :], qb_bf[:])

        for d in range(NCORES):
            qb_tile(3 * d)
            qb_tile(3 * d + 2)
        nc.gpsimd.collective_compute(
            "AllToAll", ALU.bypass, replica_groups=rg,
            ins=[q_loc0.opt()], outs=[q_mine0.opt()])
        # batched compressed-KV loads (SWDGE) right after the A2A#1 trigger:
        # 4 big DMAs instead of 32 small ones, so A2A#2 isn't starved
        ckv_sb = s2.tile([P, NKV, AT], BF16)
        kvr = kv_all.rearrange("(j r) c -> r j c", j=NCORES)
        for k in range(NKV):
            nc.gpsimd.dma_start(
                ckv_sb[:, k, :].rearrange("p (j c) -> p j c", j=NCORES),
                kvr[k * P:(k + 1) * P])
        for d in range(NCORES):
            qb_tile(3 * d + 1)
        nc.gpsimd.collective_compute(
            "AllToAll", ALU.bypass, replica_groups=rg,
            ins=[q_loc1.opt()], outs=[q_mine1.opt()])
        kro2 = s2.tile([P, AT], BF16)
        for half in range(2):
            nc.gpsimd.dma_start(
                kro2[half * DR:(half + 1) * DR, :].rearrange(
                    "p (j c) -> p j c", j=NCORES),
                kvr[KVLR:KVC])

        # stage-1 pools done
        st_s1w.close()

        # ---------------- stage 2: expand kv_b -----------------------------
        kn_sb = s2.tile([P, 2, AT], BF16)
        for h in range(2):
            for jc in range(NCORES):
                ps = ps_att.tile([P, T], F32, name="pss", tag="pss", bufs=4)
                for k in range(NKV):
                    nc.tensor.matmul(
                        ps[:], wkvb_sb[:, k, h * P:(h + 1) * P],
                        ckv_sb[:, k, jc * T:(jc + 1) * T],
                        start=(k == 0), stop=(k == NKV - 1))
                nc.vector.tensor_copy(kn_sb[:, h, jc * T:(jc + 1) * T],
                                      ps[:])

        NVT = AT // P
        v_sb = s2.tile([P, NVT, OB], BF16)
        for t in range(NVT):
            ps = ps_att.tile([P, T], F32, name="pss", tag="pss", bufs=4)[:, :OB]
            for k in range(NKV):
                nc.tensor.matmul(
                    ps[:], ckv_sb[:, k, t * P:(t + 1) * P],
                    wkvb_sb[:, k, OB:2 * OB],
                    start=(k == 0), stop=(k == NKV - 1))
            nc.vector.tensor_copy(v_sb[:, t, :], ps[:])

        # ---------------- stage 3: attention -------------------------------
        om2 = s4.tile([P, 2, NCORES, T], BF16)
        wo_tiles = []

        pending = [None]     # deferred PE consumer emission (SW pipeline)
        prs_hold = [None]    # bf16 probs accumulator across pair groups

        def flush_pending():
            if pending[0] is not None:
                pending[0]()
                pending[0] = None

        for h in range(2):
            o_loc_h = o_loc0 if h == 0 else o_loc1
            q_mine_h = q_mine0 if h == 0 else q_mine1
            for b in range(B):
                for qc in range(NBLK):
                    j = NBLK * b + qc
                    qn_sb = attq.tile([P, T], BF16, name="qn_sb")
                    nc.sync.dma_start(
                        qn_sb[:], q_mine_h[j * 192: j * 192 + P, :])
                    # load raw rope rows, rope on DVE, duplicate into both
                    # partition halves (second rope matmul runs in rows 64+)
                    qr2 = attq.tile([P, T], BF16, name="qr2", bufs=2)
                    nc.sync.dma_start(
                        qr2[0:DR, :],
                        q_mine_h[j * 192 + P: (j + 1) * 192, :])
                    nc.sync.dma_start(
                        qr2[DR:P, :],
                        q_mine_h[j * 192 + P: (j + 1) * 192, :])

                    pso = ps_att.tile([P, T], F32, name="pso", tag="pso")
                    psd = ps_att.tile([1, T], F32, name="psd", tag="psd",
                                      bufs=2)
                    nkp = 2 * (qc + 1)           # kt pairs
                    for kp in range(nkp):
                        kt0, kt1 = 2 * kp, 2 * kp + 1
                        gk0 = b * S + kt0 * P
                        gk1 = b * S + kt1 * P
                        pss_a = ps_att.tile([P, T], F32, name="pss",
                                            tag="pss", bufs=4)
                        pss_b = ps_att.tile([P, T], F32, name="pss",
                                            tag="pss", bufs=4)
                        nc.tensor.matmul(pss_a[:], kn_sb[:, h, gk0:gk0 + P],
                                         qn_sb[:], start=True, stop=False)
                        nc.tensor.matmul(pss_b[:], kn_sb[:, h, gk1:gk1 + P],
                                         qn_sb[:], start=True, stop=False)
                        # K=64 rope matmuls in different array row groups
                        nc.tensor.matmul(pss_a[:], kro2[0:DR, gk0:gk0 + P],
                                         qr2[0:DR, :], start=False,
                                         stop=True)
                        nc.tensor.matmul(pss_b[:], kro2[DR:P, gk1:gk1 + P],
                                         qr2[DR:P, :], start=False,
                                         stop=True)
                        # emit previous pair's PE consumers now — gives the
                        # ACT/DVE chain a full pair of slack
                        flush_pending()
                        pr_a = att.tile([P, T], BF16, name="pr", tag="pr",
                                        bufs=3)
                        nc.scalar.activation(pr_a[:], pss_a[:], AF.Exp,
                                             scale=SCALE)
                        dj0 = kt0 - NBLK * qc
                        if dj0 >= 0:
                            nc.vector.tensor_mul(pr_a[:], pr_a[:],
                                                 mask_sb[:, dj0, :])
                        pr_b = att.tile([P, T], BF16, name="pr", tag="pr",
                                        bufs=3)
                        nc.scalar.activation(pr_b[:], pss_b[:], AF.Exp,
                                             scale=SCALE)
                        dj1 = kt1 - NBLK * qc
                        if dj1 >= 0:
                            nc.vector.tensor_mul(pr_b[:], pr_b[:],
                                                 mask_sb[:, dj1, :])
                        if kp == 0:
                            prs = att.tile([P, T], BF16, name="prs",
                                           tag="prs", bufs=2)
                            nc.vector.tensor_add(prs[:], pr_a[:], pr_b[:])
                            prs_hold[0] = prs
                        else:
                            prs = prs_hold[0]
                            nc.vector.tensor_add(prs[:], prs[:], pr_a[:])
                            nc.vector.tensor_add(prs[:], prs[:], pr_b[:])

                        def consume(pr_a=pr_a, pr_b=pr_b, prs=prs, kp=kp,
                                    nkp=nkp, b=b, kt0=kt0, h=h, pso=pso,
                                    psd=psd, j=j, o_loc_h=o_loc_h):
                            if kp == nkp - 1:
                                nc.tensor.matmul(psd[:], ones_bf[:],
                                                 prs[:], start=True,
                                                 stop=True)
                            vt0 = (b * S) // P + kt0
                            nc.tensor.matmul(
                                pso[:], v_sb[:, vt0, h * DV:(h + 1) * DV],
                                pr_a[:], start=(kp == 0), stop=False)
                            nc.tensor.matmul(
                                pso[:],
                                v_sb[:, vt0 + 1, h * DV:(h + 1) * DV],
                                pr_b[:], start=False, stop=(kp == nkp - 1))
                            if kp == nkp - 1:
                                rec = att.tile([1, T], F32, name="rec",
                                               tag="rec", bufs=2)
                                nc.vector.reciprocal(rec[:], psd[:])
                                recb = att.tile([P, T], F32, name="recb",
                                                tag="recb", bufs=1)
                                nc.gpsimd.partition_broadcast(recb[:],
                                                              rec[:])
                                o_bf = att.tile([P, T], BF16, name="o_bf",
                                                tag="o_bf", bufs=1)
                                nc.vector.tensor_mul(o_bf[:], pso[:],
                                                     recb[:])
                                nc.sync.dma_start(
                                    o_loc_h[j * DV:(j + 1) * DV, :],
                                    o_bf[:])

                        pending[0] = consume

            flush_pending()
            # per-head A2A; the h0 one overlaps h1 attention
            o_mine_h = o_mine0 if h == 0 else o_mine1
            nc.gpsimd.collective_compute(
                "AllToAll", ALU.bypass, replica_groups=rg,
                ins=[(o_loc0 if h == 0 else o_loc1).opt()],
                outs=[o_mine_h.opt()])
            if h == 0:
                # prefetch the first 8 o_proj weight tiles on the scalar
                # queue: no deps, so the DMAs fire while h=1 attention runs
                for mh in range(8):
                    wt = s4w.tile([P, NKH, P], BF16, name="wo_t",
                                  tag="wo_t", bufs=8)
                    nc.scalar.dma_start(wt[:], wo[mh])
                    wo_tiles.append(wt)

        # gather heads into SBUF on the scalar HWDGE queue (not gpsimd, so
        # collectives/broadcasts are never stalled); emitted after all exp
        # work so the A2A-completion waits can't stall the ACT stream
        for hh in range(2):
            nc.scalar.dma_start(
                om2[:, hh, :, :],
                (o_mine0 if hh == 0 else o_mine1).rearrange(
                    "(g p) c -> p g c", g=NCORES))

        st_att.close()
        st_s2.close()

        # ---------------- stage 4: o_proj (transposed output) --------------
        # group A (first 8 hid tiles): even-head contraction runs in the
        # A2A#2 shadow (only needs o_mine0); odd heads finish after.
        NA = 6
        psA = []
        for i in range(NA):
            if i < 4:
                ps = ps_att.tile([P, T], F32, name=f"opA{i}", tag="pss",
                                 bufs=4)
            elif i < 6:
                ps = ps_att.tile([P, T], F32, name=f"opA{i}", tag="pso")
            else:
                ps = ps_att.tile([P, T], F32, name=f"opA{i}", tag="psd",
                                 bufs=2)
            psA.append(ps)
        for mh in range(NA):
            for g in range(NCORES):                   # even (h=0) heads
                nc.tensor.matmul(psA[mh][:], wo_tiles[mh][:, 2 * g, :],
                                 om2[:, 0, g, :], start=(g == 0),
                                 stop=False)

        # odd halves + eviction (runs once the odd om tiles land)
        for mh in range(NA):
            for g in range(NCORES):                   # odd (h=1) heads
                nc.tensor.matmul(psA[mh][:], wo_tiles[mh][:, 2 * g + 1, :],
                                 om2[:, 1, g, :], start=False,
                                 stop=(g == NCORES - 1))
            ot = s4t.tile([P, T], F32, name="ot")
            nc.vector.tensor_copy(ot[:], psA[mh][:])
            eng = nc.sync if mh % 2 == 0 else nc.scalar
            eng.dma_start(out[mh * P:(mh + 1) * P, :], ot[:])

        # group B: remaining hid tiles, full contraction
        for mh in range(NA, H):
            if mh >= 8:
                wt = s4w.tile([P, NKH, P], BF16, name="wo_t", tag="wo_t",
                              bufs=8)
                nc.scalar.dma_start(wt[:], wo[mh])
                wo_tiles.append(wt)
            ps = ps_att.tile([P, T], F32, name="pss", tag="pss", bufs=4)
            for k in range(NKH):
                nc.tensor.matmul(ps[:], wo_tiles[mh][:, k, :],
                                 om2[:, k % 2, k // 2, :],
                                 start=(k == 0), stop=(k == NKH - 1))
            ot = s4t.tile([P, T], F32, name="ot")
            nc.vector.tensor_copy(ot[:], ps[:])
            eng = nc.sync if mh % 2 == 0 else nc.scalar
            eng.dma_start(out[mh * P:(mh + 1) * P, :], ot[:])

    nc.compile()
    return nc


# ----------------------------------------------------------------------------
# host side: shard prep, run, gather
# ----------------------------------------------------------------------------

def _prep_in_maps(hidden_states, wq_a, gq_a, wq_b, wkv_a, gkv_a, wkv_b, wo):
    hidden_states = np.asarray(hidden_states, dtype=np.float32)
    wq_a = np.asarray(wq_a, np.float32)
    wq_b = np.asarray(wq_b, np.float32) * np.asarray(gq_a, np.float32)[:, None]
    wkv_a = np.asarray(wkv_a, np.float32)
    wkv_b = (np.asarray(wkv_b, np.float32)
             * np.asarray(gkv_a, np.float32)[:, None])
    wo = np.asarray(wo, np.float32)

    # permute wq_b columns into per-rank blocks
    perm = []
    for j in range(NCORES):
        for h in (2 * j, 2 * j + 1):
            perm.extend(range(h * DN, (h + 1) * DN))
        for h in (2 * j, 2 * j + 1):
            perm.extend(range(H * DN + h * DR, H * DN + (h + 1) * DR))
    wqb_perm = np.ascontiguousarray(wq_b[:, perm]).astype(bf16)

    def mtile(w, nk, nm):
        # [nk*128, nm*128] -> [nm, 128, nk, 128] so each SBUF weight tile
        # [p, k, m] is one contiguous DMA
        return np.ascontiguousarray(
            w.reshape(nk, P, nm, P).transpose(2, 1, 0, 3))

    wqa_b = mtile(wq_a.astype(bf16), NKH, NKQ)
    wqb_t = mtile(wqb_perm, NKQ, QCOLS // P)
    wkva_p = np.zeros((HID, 640), np.float32)
    wkva_p[:, :KVC] = wkv_a
    wkva_b = mtile(wkva_p.astype(bf16), NKH, 5)
    wo_b = mtile(wo.astype(bf16), NKH, NKH)

    inv_freq = 1.0 / (ROPE_BASE ** (np.arange(0, DR, 2, dtype=np.float32)
                                    / DR))
    masks = np.zeros((NBLK, P, T), np.float32)
    kk = np.arange(P)[:, None]
    qq = np.arange(T)[None, :]
    for jp in range(NBLK):
        masks[jp] = (P * jp + kk <= qq).astype(np.float32)
    masks_b = masks.astype(bf16)
    in_maps = []
    for c in range(NCORES):
        b, blk = divmod(c, NBLK)
        h0 = 2 * c
        tok0 = blk * T
        hT = np.ascontiguousarray(
            hidden_states[b, tok0:tok0 + T, :].T).astype(bf16)
        pos = np.arange(tok0, tok0 + T, dtype=np.float32)
        ang = inv_freq[:, None] * pos[None, :]
        cosT = np.cos(ang).astype(np.float32)
        sinT = np.sin(ang).astype(np.float32)
        cols = []
        for h in (h0, h0 + 1):
            cols.append(wkv_b[:, h * (DN + DV): h * (DN + DV) + DN])
        for h in (h0, h0 + 1):
            cols.append(wkv_b[:, h * (DN + DV) + DN: (h + 1) * (DN + DV)])
        wkvb_c = np.ascontiguousarray(
            np.concatenate(cols, 1).reshape(NKV, P, 512).transpose(1, 0, 2)
        ).astype(bf16)
        in_maps.append({
            "hT": hT, "wqa": wqa_b, "wqb": wqb_t, "wkva": wkva_b,
            "wkvb": wkvb_c, "wo": wo_b,
            "cst": np.concatenate([cosT, sinT], 0).astype(bf16),
            "masks": masks_b,
        })
    return in_maps


_NC_CACHE = {}


def _install_profile_hook():
    """The agent image's antenv lacks axon_hooks; recreate it so
    run_bass_kernel_spmd(trace=True) can capture NTFF profiles."""
    import sys
    import types
    if "antenv.axon_hooks" in sys.modules:
        return
    mod = types.ModuleType("antenv.axon_hooks")
    mod._hook = None

    def set_axon_ntff_profile_hook(h):
        mod._hook = h

    def get_axon_ntff_profile_hook():
        return mod._hook

    mod.set_axon_ntff_profile_hook = set_axon_ntff_profile_hook
    mod.get_axon_ntff_profile_hook = get_axon_ntff_profile_hook
    sys.modules["antenv.axon_hooks"] = mod
    try:
        import antenv
        antenv.axon_hooks = mod
        from trn_agent_boot.trn_boot import _ntff_profile_via_ctypes
        hook = _ntff_profile_via_ctypes("/opt/axon/libaxon_pjrt.so")
        if hook is not None:
            mod._hook = hook
    except Exception as e:  # degrade to no tracing
        print(f"profile hook install failed: {e}")


def _get_nc():
    if "nc" not in _NC_CACHE:
        _NC_CACHE["nc"] = _build_program()
    return _NC_CACHE["nc"]


def run(inputs, trace=False):
    if trace:
        _install_profile_hook()
    nc = _get_nc()
    in_maps = _prep_in_maps(**inputs)
    res = bass_utils.run_bass_kernel_spmd(
        nc, in_maps, core_ids=list(range(NCORES)), trace=trace)
    full = np.zeros((B, S, HID), np.float32)
    for c in range(NCORES):
        b, blk = divmod(c, NBLK)
        full[b, blk * T:(blk + 1) * T, :] = res.results[c]["out"].T
    return full, res


def kernel(**inputs) -> np.ndarray:
    full, _ = run(inputs, trace=False)
    return full



# revision 22
# speedup vs baseline: 1.0222x; 1.0181x over previous
"""DeepseekV3 MLA attention on 8 trn2 NeuronCores.

Sharding: core c owns token shard (batch c//4, seq block c%4 of 512) for the
low-rank projections and the final o_proj, and heads {2c, 2c+1} for the
attention itself.  Communication: one AllGather of the compressed KV latent,
one AllToAll to redistribute q from token-shards to head-shards, and one
AllToAll to bring attention outputs back to token-shards before o_proj.
All matmuls run bf16 with fp32 PSUM accumulation.
"""

import contextlib
import math
import numpy as np
import ml_dtypes

import concourse.bacc as bacc
import concourse.bass as bass
import concourse.mybir as mybir
import concourse.tile as tile
from concourse import bass_utils

F32 = mybir.dt.float32
BF16 = mybir.dt.bfloat16
AF = mybir.ActivationFunctionType
ALU = mybir.AluOpType

# ---- model dims (hardcoded per problem spec) ----
B, S, HID = 2, 2048, 2048
H = 16
QLR, KVLR = 1536, 512
DN, DR, DV = 128, 64, 128
EPS = 1e-6
ROPE_BASE = 10000.0
SCALE = 1.0 / math.sqrt(DN + DR)

NCORES = 8
T = (B * S) // NCORES          # 512 tokens per core
NBLK = S // T                  # 4 seq blocks per batch
P = 128
QCOLS = H * (DN + DR)          # 3072
RB = QCOLS // NCORES           # 384 q rows per rank block (perm layout)
KVC = KVLR + DR                # 576
OB = 2 * DV                    # 256 rows per rank block of o_loc
AT = B * S                     # 4096 tokens total
NKH = HID // P                 # 16
NKQ = QLR // P                 # 12
NKV = KVLR // P                # 4

bf16 = ml_dtypes.bfloat16


# ----------------------------------------------------------------------------
# device program
# ----------------------------------------------------------------------------

def _rope(nc, pool, dst_bf, src, cos_sb, sin_sb, nheads):
    hw = DR // 2
    for h in range(nheads):
        x1 = src[h * DR: h * DR + hw, :]
        x2 = src[h * DR + hw: (h + 1) * DR, :]
        ta = pool.tile([hw, T], F32, name="rope_ta", tag="sq", bufs=2)
        tb = pool.tile([hw, T], F32, name="rope_tb", tag="sq", bufs=2)
        nc.vector.tensor_mul(ta[:], x1, cos_sb[:])
        nc.vector.tensor_mul(tb[:], x2, sin_sb[:])
        nc.vector.tensor_sub(dst_bf[h * DR: h * DR + hw, :], ta[:], tb[:])
        nc.vector.tensor_mul(ta[:], x1, sin_sb[:])
        nc.vector.tensor_mul(tb[:], x2, cos_sb[:])
        nc.vector.tensor_add(dst_bf[h * DR + hw: (h + 1) * DR, :], ta[:], tb[:])


def _build_program():
    nc = bacc.Bacc("TRN2", target_bir_lowering=False, debug=False,
                   num_devices=NCORES)

    def din(name, shape, dt=BF16):
        return nc.dram_tensor(name, list(shape), dt, kind="ExternalInput").ap()

    hT = din("hT", [HID, T])
    # weights host-pre-tiled: [m_tile, p, k_tile, m_cols] contiguous so each
    # SBUF weight tile is a single contiguous DMA
    wqa = din("wqa", [NKQ, P, NKH, P])
    wqb = din("wqb", [QCOLS // P, P, NKQ, P])
    wkva = din("wkva", [5, P, NKH, P])
    wkvb = din("wkvb", [P, NKV, 512])
    wo = din("wo", [NKH, P, NKH, P])
    cst = din("cst", [DR, T])                # bf16 cos/sin rows (own pos)
    masks = din("masks", [NBLK, P, T])
    # out is feature-major [HID, T]; host transposes
    out = nc.dram_tensor("out", [HID, T], F32, kind="ExternalOutput").ap()
    rg = [list(range(NCORES))]

    ctx = contextlib.ExitStack()
    with tile.TileContext(nc) as tc, ctx:
        dram = ctx.enter_context(tc.tile_pool(name="dram", bufs=1,
                                              space="DRAM"))
        kv_loc = dram.tile([KVC, T], BF16)
        kv_all = dram.tile([NCORES * KVC, T], BF16, addr_space="Shared")
        q_loc0 = dram.tile([NCORES * 192, T], BF16)
        q_loc1 = dram.tile([NCORES * 192, T], BF16)
        q_mine0 = dram.tile([NCORES * 192, T], BF16)
        q_mine1 = dram.tile([NCORES * 192, T], BF16)
        o_loc0 = dram.tile([NCORES * DV, T], BF16)
        o_loc1 = dram.tile([NCORES * DV, T], BF16)
        o_mine0 = dram.tile([NCORES * DV, T], BF16)
        o_mine1 = dram.tile([NCORES * DV, T], BF16)

        # ---- LEFT stack: long-lived pools --------------------------------
        const = ctx.enter_context(tc.tile_pool(name="const", bufs=1,
                                               side="left"))
        s4 = ctx.enter_context(tc.tile_pool(name="s4", bufs=1, side="left"))
        s4w = ctx.enter_context(tc.tile_pool(name="s4w", bufs=3,
                                             side="left"))
        s4t = ctx.enter_context(tc.tile_pool(name="s4t", bufs=1,
                                             side="left"))
        ps_att = ctx.enter_context(tc.tile_pool(name="ps_att", bufs=2,
                                                space="PSUM", side="left"))
        st_s2 = contextlib.ExitStack()
        s2 = st_s2.enter_context(tc.tile_pool(name="s2", bufs=1,
                                              side="left"))
        st_att = contextlib.ExitStack()
        att = st_att.enter_context(tc.tile_pool(name="att", bufs=1,
                                                side="left"))
        attq = st_att.enter_context(tc.tile_pool(name="attq", bufs=2,
                                                 side="left"))

        # ---- RIGHT stack: stage-1 pools (released mid-kernel) ------------
        st_s1w = contextlib.ExitStack()
        pool_qlr = st_s1w.enter_context(tc.tile_pool(name="pool_qlr",
                                                     bufs=1, side="right"))
        s1w = st_s1w.enter_context(tc.tile_pool(name="s1w", bufs=2,
                                                side="right"))
        s1wb = st_s1w.enter_context(tc.tile_pool(name="s1wb", bufs=2,
                                                 side="right"))
        s1t = st_s1w.enter_context(tc.tile_pool(name="s1t", bufs=1,
                                                side="right"))
        st_h = contextlib.ExitStack()
        ph = st_h.enter_context(tc.tile_pool(name="pool_h", bufs=1,
                                             side="right"))

        # ---- consts; hT first (PE needs it immediately) ------------------
        hT_sb = ph.tile([P, NKH, T], BF16)
        for k in range(NKH):
            nc.scalar.dma_start(hT_sb[:, k, :], hT[k * P:(k + 1) * P, :])
        ones_bf = const.tile([P, 1], BF16)
        nc.gpsimd.memset(ones_bf[:], 1.0)
        CC_sb = const.tile([P, T], BF16)   # [cos;cos;cos;cos] rows
        SS_sb = const.tile([P, T], BF16)
        hw = DR // 2
        for r in range(4):
            nc.scalar.dma_start(CC_sb[r * hw:(r + 1) * hw, :],
                                cst[0:hw, :])
            nc.scalar.dma_start(SS_sb[r * hw:(r + 1) * hw, :],
                                cst[hw:DR, :])
        cos_sb = CC_sb[0:hw, :]
        sin_sb = SS_sb[0:hw, :]
        mask_sb = const.tile([P, NBLK, T], BF16)
        for j in range(NBLK):
            nc.scalar.dma_start(mask_sb[:, j, :], masks[j])

        qlr_bf = pool_qlr.tile([P, NKQ, T], BF16)
        kv_bf = pool_qlr.tile([P, NKV, T], BF16)

        # wkvb is tiny and dep-free: load early on the scalar queue
        wkvb_sb = s2.tile([P, NKV, 512], BF16)
        nc.scalar.dma_start(wkvb_sb[:], wkvb)

        # ---------------- stage 1a: kv_a + rmsnorm + rope ------------------
        # m-tiles processed in PAIRS with interleaved PE emission: two
        # independent PSUM accumulation chains let the PE hide stationary
        # weight loads behind the other chain's streaming.
        ss_kv = ps_att.tile([1, T], F32, name="ss_kv", tag="psd", bufs=2)
        kv_m_sizes = [P] * NKV + [DR]

        def s1_pair(wsrc, ms, nk, mov, mw_of=None):
            wts, pss = [], []
            for m in ms:
                wt = s1w.tile([P, nk, P], BF16, name="w_t", tag="w_t",
                              bufs=3)
                nc.sync.dma_start(wt[:], wsrc[m])
                wts.append(wt)
                pss.append(ps_att.tile([P, T], F32, name="pss", tag="pss",
                                       bufs=4))
            for k in range(nk):
                for i, m in enumerate(ms):
                    mw = P if mw_of is None else mw_of[m]
                    nc.tensor.matmul(pss[i][:mw, :], wts[i][:, k, :mw],
                                     mov[:, k, :], start=(k == 0),
                                     stop=(k == nk - 1))
            return pss

        def kva_evict(m, ps):
            if m < NKV:
                nc.vector.tensor_copy(kv_bf[:, m, :], ps[:, :])
                sq = s1t.tile([P, T], BF16, name="sq", tag="sq", bufs=2)
                nc.vector.tensor_mul(sq[:], kv_bf[:, m, :], kv_bf[:, m, :])
                nc.tensor.matmul(ss_kv[:], ones_bf[:], sq[:],
                                 start=(m == 0), stop=(m == NKV - 1))
            else:
                kro_bf = s1t.tile([DR, T], BF16, name="kro_bf", tag="obf",
                                  bufs=2)
                _rope(nc, s1t, kro_bf[:], ps, cos_sb, sin_sb, 1)
                nc.scalar.dma_start(kv_loc[KVLR:KVC, :], kro_bf[:])

        for ms in ((0, 1), (2, 3), (4,)):
            pss = s1_pair(wkva, ms, NKH, hT_sb, kv_m_sizes)
            for i, m in enumerate(ms):
                kva_evict(m, pss[i])

        tmp = s1t.tile([1, T], F32, name="tmp_kv", tag="tmp", bufs=1)
        nc.vector.tensor_scalar(tmp[:], ss_kv[:], 1.0 / KVLR, EPS,
                                op0=ALU.mult, op1=ALU.add)
        nc.scalar.activation(tmp[:], tmp[:], AF.Sqrt)
        nc.vector.reciprocal(tmp[:], tmp[:])
        rsb_kv = s1t.tile([P, T], F32, name="rsb_kv", tag="rsb", bufs=2)
        nc.gpsimd.partition_broadcast(rsb_kv[:], tmp[:])
        for m in range(NKV):
            ckv_bf = s1t.tile([P, T], BF16, name="ckv_bf", tag="obf", bufs=2)
            nc.vector.tensor_mul(ckv_bf[:], kv_bf[:, m, :], rsb_kv[:])
            nc.scalar.dma_start(kv_loc[m * P:(m + 1) * P, :], ckv_bf[:])

        nc.gpsimd.collective_compute(
            "AllGather", ALU.bypass, replica_groups=rg,
            ins=[kv_loc.opt()], outs=[kv_all.opt()])

        # ---------------- stage 1b: q_a + rmsnorm --------------------------
        ss_q = ps_att.tile([1, T], F32, name="ss_q", tag="psd", bufs=2)
        for mp in range(0, NKQ, 2):
            pss = s1_pair(wqa, (mp, mp + 1), NKH, hT_sb)
            for i, m in enumerate((mp, mp + 1)):
                nc.vector.tensor_copy(qlr_bf[:, m, :], pss[i][:])
                sq = s1t.tile([P, T], BF16, name="sq", tag="sq", bufs=2)
                nc.vector.tensor_mul(sq[:], qlr_bf[:, m, :],
                                     qlr_bf[:, m, :])
                nc.tensor.matmul(ss_q[:], ones_bf[:], sq[:],
                                 start=(m == 0), stop=(m == NKQ - 1))

        tmpq = s1t.tile([1, T], F32, name="tmp_q", tag="tmp", bufs=1)
        nc.vector.tensor_scalar(tmpq[:], ss_q[:], 1.0 / QLR, EPS,
                                op0=ALU.mult, op1=ALU.add)
        nc.scalar.activation(tmpq[:], tmpq[:], AF.Sqrt)
        nc.vector.reciprocal(tmpq[:], tmpq[:])
        rsb_q = s1t.tile([P, T], F32, name="rsb_q", tag="rsb", bufs=2)
        nc.gpsimd.partition_broadcast(rsb_q[:], tmpq[:])

        # hT no longer needed
        st_h.close()

        # ---------------- stage 1c: q_b (+rope), permuted ------------------
        # unnormalized q_lr feeds the matmul; the per-token norm scale is
        # multiplied into the outputs (commutes through matmul and rope).
        # m-tile order: even-head halves first so the first A2A fires early.
        def qb_pair(ms):
            wts, pss = [], []
            for m in ms:
                wt = s1wb.tile([P, NKQ, P], BF16, name="wqb_t",
                               tag="wqb_t", bufs=3)
                nc.sync.dma_start(wt[:], wqb[m])
                wts.append(wt)
                pss.append(ps_att.tile([P, T], F32, name="pss", tag="pss",
                                       bufs=4))
            for k in range(NKQ):
                for i in range(len(ms)):
                    nc.tensor.matmul(pss[i][:], wts[i][:, k, :],
                                     qlr_bf[:, k, :], start=(k == 0),
                                     stop=(k == NKQ - 1))
            for i, m in enumerate(ms):
                qb_evict(m, pss[i])

        def qb_evict(m, ps):
            d = m // 3
            qb_bf = s1t.tile([P, T], BF16, name="qb_bf", tag="obf", bufs=2)
            if m % 3 == 2:
                hw = DR // 2
                xr = s1t.tile([P, T], F32, name="xr", tag="sq", bufs=2)
                for hh in range(2):
                    o = hh * DR
                    nc.vector.tensor_scalar_mul(
                        xr[o:o + hw, :], ps[o + hw:o + DR, :], -1.0)
                    nc.vector.tensor_copy(xr[o + hw:o + DR, :],
                                          ps[o:o + hw, :])
                t1 = s1t.tile([P, T], F32, name="t1", tag="rsb", bufs=2)
                nc.vector.tensor_mul(t1[:], ps[:], CC_sb[:])
                t2 = s1t.tile([P, T], F32, name="t2", tag="sq", bufs=2)
                nc.vector.tensor_mul(t2[:], xr[:], SS_sb[:])
                nc.vector.tensor_add(t1[:], t1[:], t2[:])
                nc.vector.tensor_mul(qb_bf[:], t1[:], rsb_q[:])
                nc.scalar.dma_start(
                    q_loc0[d * 192 + P: (d + 1) * 192, :], qb_bf[0:DR, :])
                nc.scalar.dma_start(
                    q_loc1[d * 192 + P: (d + 1) * 192, :], qb_bf[DR:P, :])
            else:
                nc.vector.tensor_mul(qb_bf[:], ps[:], rsb_q[:])
                qdst = q_loc0 if m % 3 == 0 else q_loc1
                nc.scalar.dma_start(qdst[d * 192: d * 192 + P, :], qb_bf[:])

        for d in range(NCORES):
            qb_pair((3 * d, 3 * d + 2))
        nc.gpsimd.collective_compute(
            "AllToAll", ALU.bypass, replica_groups=rg,
            ins=[q_loc0.opt()], outs=[q_mine0.opt()])
        # batched compressed-KV loads (SWDGE) right after the A2A#1 trigger:
        # 4 big DMAs instead of 32 small ones, so A2A#2 isn't starved
        ckv_sb = s2.tile([P, NKV, AT], BF16)
        kvr = kv_all.rearrange("(j r) c -> r j c", j=NCORES)
        for k in range(NKV):
            nc.gpsimd.dma_start(
                ckv_sb[:, k, :].rearrange("p (j c) -> p j c", j=NCORES),
                kvr[k * P:(k + 1) * P])
        for dp in range(0, NCORES, 2):
            qb_pair((3 * dp + 1, 3 * (dp + 1) + 1))
        nc.gpsimd.collective_compute(
            "AllToAll", ALU.bypass, replica_groups=rg,
            ins=[q_loc1.opt()], outs=[q_mine1.opt()])
        kro2 = s2.tile([P, AT], BF16)
        for half in range(2):
            nc.gpsimd.dma_start(
                kro2[half * DR:(half + 1) * DR, :].rearrange(
                    "p (j c) -> p j c", j=NCORES),
                kvr[KVLR:KVC])

        # stage-1 pools done
        st_s1w.close()

        # ---------------- stage 2: expand kv_b -----------------------------
        # dual interleaved chains; evictions on gpsimd so the DVE stays free
        # for the attention chain that starts right behind this
        kn_sb = s2.tile([P, 2, AT], BF16)
        for jc in range(NCORES):
            pss = [ps_att.tile([P, T], F32, name="pss", tag="pss", bufs=4)
                   for _ in range(2)]
            for k in range(NKV):
                for h in range(2):
                    nc.tensor.matmul(
                        pss[h][:], wkvb_sb[:, k, h * P:(h + 1) * P],
                        ckv_sb[:, k, jc * T:(jc + 1) * T],
                        start=(k == 0), stop=(k == NKV - 1))
            for h in range(2):
                nc.scalar.copy(kn_sb[:, h, jc * T:(jc + 1) * T],
                               pss[h][:])

        NVT = AT // P
        v_sb = s2.tile([P, NVT, OB], BF16)
        for t0 in range(0, NVT, 2):
            pss = [ps_att.tile([P, T], F32, name="pss", tag="pss",
                               bufs=4)[:, :OB] for _ in range(2)]
            for k in range(NKV):
                for i in range(2):
                    nc.tensor.matmul(
                        pss[i][:], ckv_sb[:, k, (t0 + i) * P:(t0 + i + 1) * P],
                        wkvb_sb[:, k, OB:2 * OB],
                        start=(k == 0), stop=(k == NKV - 1))
            for i in range(2):
                nc.scalar.copy(v_sb[:, t0 + i, :], pss[i][:])

        # ---------------- stage 3: attention -------------------------------
        om2 = s4.tile([P, 2, NCORES, T], BF16)
        wo_tiles = []

        pending = [None]     # deferred PE consumer emission (SW pipeline)
        prs_hold = [None]    # bf16 probs accumulator across pair groups

        def flush_pending():
            if pending[0] is not None:
                pending[0]()
                pending[0] = None

        for h in range(2):
            o_loc_h = o_loc0 if h == 0 else o_loc1
            q_mine_h = q_mine0 if h == 0 else q_mine1
            for b in range(B):
                for qc in range(NBLK):
                    j = NBLK * b + qc
                    qn_sb = attq.tile([P, T], BF16, name="qn_sb")
                    nc.sync.dma_start(
                        qn_sb[:], q_mine_h[j * 192: j * 192 + P, :])
                    # load raw rope rows, rope on DVE, duplicate into both
                    # partition halves (second rope matmul runs in rows 64+)
                    qr2 = attq.tile([P, T], BF16, name="qr2", bufs=2)
                    nc.sync.dma_start(
                        qr2[0:DR, :],
                        q_mine_h[j * 192 + P: (j + 1) * 192, :])
                    nc.sync.dma_start(
                        qr2[DR:P, :],
                        q_mine_h[j * 192 + P: (j + 1) * 192, :])

                    pso = ps_att.tile([P, T], F32, name="pso", tag="pso")
                    psd = ps_att.tile([1, T], F32, name="psd", tag="psd",
                                      bufs=2)
                    nkp = 2 * (qc + 1)           # kt pairs
                    for kp in range(nkp):
                        kt0, kt1 = 2 * kp, 2 * kp + 1
                        gk0 = b * S + kt0 * P
                        gk1 = b * S + kt1 * P
                        pss_a = ps_att.tile([P, T], F32, name="pss",
                                            tag="pss", bufs=4)
                        pss_b = ps_att.tile([P, T], F32, name="pss",
                                            tag="pss", bufs=4)
                        nc.tensor.matmul(pss_a[:], kn_sb[:, h, gk0:gk0 + P],
                                         qn_sb[:], start=True, stop=False)
                        nc.tensor.matmul(pss_b[:], kn_sb[:, h, gk1:gk1 + P],
                                         qn_sb[:], start=True, stop=False)
                        # K=64 rope matmuls in different array row groups
                        nc.tensor.matmul(pss_a[:], kro2[0:DR, gk0:gk0 + P],
                                         qr2[0:DR, :], start=False,
                                         stop=True)
                        nc.tensor.matmul(pss_b[:], kro2[DR:P, gk1:gk1 + P],
                                         qr2[DR:P, :], start=False,
                                         stop=True)
                        # emit previous pair's PE consumers now — gives the
                        # ACT/DVE chain a full pair of slack
                        flush_pending()
                        pr_a = att.tile([P, T], BF16, name="pr", tag="pr",
                                        bufs=3)
                        nc.scalar.activation(pr_a[:], pss_a[:], AF.Exp,
                                             scale=SCALE)
                        dj0 = kt0 - NBLK * qc
                        if dj0 >= 0:
                            nc.vector.tensor_mul(pr_a[:], pr_a[:],
                                                 mask_sb[:, dj0, :])
                        pr_b = att.tile([P, T], BF16, name="pr", tag="pr",
                                        bufs=3)
                        nc.scalar.activation(pr_b[:], pss_b[:], AF.Exp,
                                             scale=SCALE)
                        dj1 = kt1 - NBLK * qc
                        if dj1 >= 0:
                            nc.vector.tensor_mul(pr_b[:], pr_b[:],
                                                 mask_sb[:, dj1, :])
                        if kp == 0:
                            prs = att.tile([P, T], BF16, name="prs",
                                           tag="prs", bufs=2)
                            nc.vector.tensor_add(prs[:], pr_a[:], pr_b[:])
                            prs_hold[0] = prs
                        else:
                            prs = prs_hold[0]
                            nc.vector.tensor_add(prs[:], prs[:], pr_a[:])
                            nc.vector.tensor_add(prs[:], prs[:], pr_b[:])

                        def consume(pr_a=pr_a, pr_b=pr_b, prs=prs, kp=kp,
                                    nkp=nkp, b=b, kt0=kt0, h=h, pso=pso,
                                    psd=psd, j=j, o_loc_h=o_loc_h):
                            if kp == nkp - 1:
                                nc.tensor.matmul(psd[:], ones_bf[:],
                                                 prs[:], start=True,
                                                 stop=True)
                            vt0 = (b * S) // P + kt0
                            nc.tensor.matmul(
                                pso[:], v_sb[:, vt0, h * DV:(h + 1) * DV],
                                pr_a[:], start=(kp == 0), stop=False)
                            nc.tensor.matmul(
                                pso[:],
                                v_sb[:, vt0 + 1, h * DV:(h + 1) * DV],
                                pr_b[:], start=False, stop=(kp == nkp - 1))
                            if kp == nkp - 1:
                                den_row = att.tile([1, T], F32,
                                                   name="den_row",
                                                   tag="rec", bufs=2)
                                nc.scalar.copy(den_row[:], psd[:])
                                den_sq = att.tile([P, T // P], F32,
                                                  name="den_sq",
                                                  tag="den_sq", bufs=2)
                                nc.sync.dma_start(den_sq[:], den_row[:])
                                nc.vector.reciprocal(den_sq[:], den_sq[:])
                                rec = att.tile([1, T], F32, name="rec",
                                               tag="rec", bufs=2)
                                nc.sync.dma_start(rec[:], den_sq[:])
                                recb = att.tile([P, T], F32, name="recb",
                                                tag="recb", bufs=1)
                                nc.gpsimd.partition_broadcast(recb[:],
                                                              rec[:])
                                o_bf = att.tile([P, T], BF16, name="o_bf",
                                                tag="o_bf", bufs=1)
                                nc.vector.tensor_mul(o_bf[:], pso[:],
                                                     recb[:])
                                nc.sync.dma_start(
                                    o_loc_h[j * DV:(j + 1) * DV, :],
                                    o_bf[:])

                        pending[0] = consume

            flush_pending()
            # per-head A2A; the h0 one overlaps h1 attention
            o_mine_h = o_mine0 if h == 0 else o_mine1
            nc.gpsimd.collective_compute(
                "AllToAll", ALU.bypass, replica_groups=rg,
                ins=[(o_loc0 if h == 0 else o_loc1).opt()],
                outs=[o_mine_h.opt()])
            if h == 0:
                # prefetch the first 6 o_proj weight tiles on the scalar
                # queue: no deps, so the DMAs fire while h=1 attention runs
                for mh in range(6):
                    wt = s4w.tile([P, NKH, P], BF16, name="wo_t",
                                  tag="wo_t", bufs=6)
                    nc.scalar.dma_start(wt[:], wo[mh])
                    wo_tiles.append(wt)

        # gather heads into SBUF on the scalar HWDGE queue (not gpsimd, so
        # collectives/broadcasts are never stalled); emitted after all exp
        # work so the A2A-completion waits can't stall the ACT stream
        for hh in range(2):
            nc.scalar.dma_start(
                om2[:, hh, :, :],
                (o_mine0 if hh == 0 else o_mine1).rearrange(
                    "(g p) c -> p g c", g=NCORES))

        st_att.close()
        st_s2.close()

        # ---------------- stage 4: o_proj (transposed output) --------------
        # group A (first 8 hid tiles): even-head contraction runs in the
        # A2A#2 shadow (only needs o_mine0); odd heads finish after.
        NA = 6
        psA = []
        for i in range(NA):
            if i < 4:
                ps = ps_att.tile([P, T], F32, name=f"opA{i}", tag="pss",
                                 bufs=4)
            elif i < 6:
                ps = ps_att.tile([P, T], F32, name=f"opA{i}", tag="pso")
            else:
                ps = ps_att.tile([P, T], F32, name=f"opA{i}", tag="psd",
                                 bufs=2)
            psA.append(ps)
        for mp in range(0, NA, 2):
            for g in range(NCORES):                   # even (h=0) heads
                for mh in (mp, mp + 1):
                    nc.tensor.matmul(psA[mh][:],
                                     wo_tiles[mh][:, 2 * g, :],
                                     om2[:, 0, g, :], start=(g == 0),
                                     stop=False)

        # odd halves + eviction (runs once the odd om tiles land)
        for mp in range(0, NA, 2):
            for g in range(NCORES):                   # odd (h=1) heads
                for mh in (mp, mp + 1):
                    nc.tensor.matmul(psA[mh][:],
                                     wo_tiles[mh][:, 2 * g + 1, :],
                                     om2[:, 1, g, :], start=False,
                                     stop=(g == NCORES - 1))
            for mh in (mp, mp + 1):
                ot = s4t.tile([P, T], F32, name="ot")
                nc.vector.tensor_copy(ot[:], psA[mh][:])
                eng = nc.sync if mh % 2 == 0 else nc.scalar
                eng.dma_start(out[mh * P:(mh + 1) * P, :], ot[:])

        # group B: remaining hid tiles, full contraction, paired chains
        for mp in range(NA, H, 2):
            pss = []
            for mh in (mp, mp + 1):
                wt = s4w.tile([P, NKH, P], BF16, name="wo_t", tag="wo_t",
                              bufs=6)
                nc.scalar.dma_start(wt[:], wo[mh])
                wo_tiles.append(wt)
                pss.append(ps_att.tile([P, T], F32, name="pss", tag="pss",
                                       bufs=4))
            for k in range(NKH):
                for i in range(2):
                    nc.tensor.matmul(pss[i][:], wo_tiles[mp + i][:, k, :],
                                     om2[:, k % 2, k // 2, :],
                                     start=(k == 0), stop=(k == NKH - 1))
            for i, mh in enumerate((mp, mp + 1)):
                ot = s4t.tile([P, T], F32, name="ot")
                nc.vector.tensor_copy(ot[:], pss[i][:])
                eng = nc.sync if mh % 2 == 0 else nc.scalar
                eng.dma_start(out[mh * P:(mh + 1) * P, :], ot[:])

    nc.compile()
    return nc


# ----------------------------------------------------------------------------
# host side: shard prep, run, gather
# ----------------------------------------------------------------------------

def _prep_in_maps(hidden_states, wq_a, gq_a, wq_b, wkv_a, gkv_a, wkv_b, wo):
    hidden_states = np.asarray(hidden_states, dtype=np.float32)
    wq_a = np.asarray(wq_a, np.float32)
    wq_b = np.asarray(wq_b, np.float32) * np.asarray(gq_a, np.float32)[:, None]
    wkv_a = np.asarray(wkv_a, np.float32)
    wkv_b = (np.asarray(wkv_b, np.float32)
             * np.asarray(gkv_a, np.float32)[:, None])
    wo = np.asarray(wo, np.float32)

    # permute wq_b columns into per-rank blocks
    perm = []
    for j in range(NCORES):
        for h in (2 * j, 2 * j + 1):
            perm.extend(range(h * DN, (h + 1) * DN))
        for h in (2 * j, 2 * j + 1):
            perm.extend(range(H * DN + h * DR, H * DN + (h + 1) * DR))
    wqb_perm = np.ascontiguousarray(wq_b[:, perm]).astype(bf16)

    def mtile(w, nk, nm):
        # [nk*128, nm*128] -> [nm, 128, nk, 128] so each SBUF weight tile
        # [p, k, m] is one contiguous DMA
        return np.ascontiguousarray(
            w.reshape(nk, P, nm, P).transpose(2, 1, 0, 3))

    wqa_b = mtile(wq_a.astype(bf16), NKH, NKQ)
    wqb_t = mtile(wqb_perm, NKQ, QCOLS // P)
    wkva_p = np.zeros((HID, 640), np.float32)
    wkva_p[:, :KVC] = wkv_a
    wkva_b = mtile(wkva_p.astype(bf16), NKH, 5)
    wo_b = mtile(wo.astype(bf16), NKH, NKH)

    inv_freq = 1.0 / (ROPE_BASE ** (np.arange(0, DR, 2, dtype=np.float32)
                                    / DR))
    masks = np.zeros((NBLK, P, T), np.float32)
    kk = np.arange(P)[:, None]
    qq = np.arange(T)[None, :]
    for jp in range(NBLK):
        masks[jp] = (P * jp + kk <= qq).astype(np.float32)
    masks_b = masks.astype(bf16)
    in_maps = []
    for c in range(NCORES):
        b, blk = divmod(c, NBLK)
        h0 = 2 * c
        tok0 = blk * T
        hT = np.ascontiguousarray(
            hidden_states[b, tok0:tok0 + T, :].T).astype(bf16)
        pos = np.arange(tok0, tok0 + T, dtype=np.float32)
        ang = inv_freq[:, None] * pos[None, :]
        cosT = np.cos(ang).astype(np.float32)
        sinT = np.sin(ang).astype(np.float32)
        cols = []
        for h in (h0, h0 + 1):
            cols.append(wkv_b[:, h * (DN + DV): h * (DN + DV) + DN])
        for h in (h0, h0 + 1):
            cols.append(wkv_b[:, h * (DN + DV) + DN: (h + 1) * (DN + DV)])
        wkvb_c = np.ascontiguousarray(
            np.concatenate(cols, 1).reshape(NKV, P, 512).transpose(1, 0, 2)
        ).astype(bf16)
        in_maps.append({
            "hT": hT, "wqa": wqa_b, "wqb": wqb_t, "wkva": wkva_b,
            "wkvb": wkvb_c, "wo": wo_b,
            "cst": np.concatenate([cosT, sinT], 0).astype(bf16),
            "masks": masks_b,
        })
    return in_maps


_NC_CACHE = {}


def _install_profile_hook():
    """The agent image's antenv lacks axon_hooks; recreate it so
    run_bass_kernel_spmd(trace=True) can capture NTFF profiles."""
    import sys
    import types
    if "antenv.axon_hooks" in sys.modules:
        return
    mod = types.ModuleType("antenv.axon_hooks")
    mod._hook = None

    def set_axon_ntff_profile_hook(h):
        mod._hook = h

    def get_axon_ntff_profile_hook():
        return mod._hook

    mod.set_axon_ntff_profile_hook = set_axon_ntff_profile_hook
    mod.get_axon_ntff_profile_hook = get_axon_ntff_profile_hook
    sys.modules["antenv.axon_hooks"] = mod
    try:
        import antenv
        antenv.axon_hooks = mod
        from trn_agent_boot.trn_boot import _ntff_profile_via_ctypes
        hook = _ntff_profile_via_ctypes("/opt/axon/libaxon_pjrt.so")
        if hook is not None:
            mod._hook = hook
    except Exception as e:  # degrade to no tracing
        print(f"profile hook install failed: {e}")


def _get_nc():
    if "nc" not in _NC_CACHE:
        _NC_CACHE["nc"] = _build_program()
    return _NC_CACHE["nc"]


def run(inputs, trace=False):
    if trace:
        _install_profile_hook()
    nc = _get_nc()
    in_maps = _prep_in_maps(**inputs)
    res = bass_utils.run_bass_kernel_spmd(
        nc, in_maps, core_ids=list(range(NCORES)), trace=trace)
    full = np.zeros((B, S, HID), np.float32)
    for c in range(NCORES):
        b, blk = divmod(c, NBLK)
        full[b, blk * T:(blk + 1) * T, :] = res.results[c]["out"].T
    return full, res


def kernel(**inputs) -> np.ndarray:
    full, _ = run(inputs, trace=False)
    return full

